# revision 1
# baseline (speedup 1.0000x reference)
"""Multi-head attention TRN2 Bass kernel, sharded over 8 NeuronCores.

Sharding: batch (2) x head-groups (4 heads each) -> 8 cores.
Each core computes QKV projection for its 4 heads, attention, and a partial
output projection restricted to its heads' columns; the host sums the 4
partials per batch and adds the (bias) terms.

All matmuls run in fp32r (TRN2 fast fp32 mode: RNE to 11-bit mantissa
operands, fp32 accumulate) — measured ~1.5e-4 rel err per matmul.

Layouts (per core):
  xT   [1024, 2048]  x[b].T                      (fp32r)
  wqT  [1024, 256]   (qkv_w q-rows * scale).T    (fp32r)
  wkT  [1024, 256]   qkv_w k-rows .T             (fp32r)
  wvT  [1024, 256]   qkv_w v-rows .T             (fp32r)
  bq   [128, 2]      q bias, partition layout    (fp32)
  pwT  [256, 1024]   proj_w[:, head-cols].T      (fp32r)
  out  [2048, 1024]  partial projection          (fp32)

On-chip: QT/KT [256, 2048] transposed (head-dim on partitions), V' [nk, 65]
per head with a ones column (its matmul row doubles as the softmax
denominator), scores computed transposed S_T[nk, nq] so softmax reduction
comes free from the ones row, exp on ACT (PSUM -> SBUF), attnV accumulated
over nk chunks in PSUM, normalization by DVE with a GPSIMD
partition-broadcast reciprocal. K-bias is softmax-invariant (dropped);
V-bias passes through softmax exactly (added on host via proj_w @ bv).
"""
import sys

sys.path.insert(0, "/opt/trn_rl_repo")

import numpy as np

import concourse.bass as bass  # noqa: F401  (engine types)
import concourse.mybir as mybir
import concourse.tile as tile
from concourse import bacc
from concourse.bass_utils import run_bass_kernel_spmd

F32 = mybir.dt.float32
F32R = mybir.dt.float32r

B = 2
N = 2048
C = 1024
HEADS = 16
HD = 64
NC_CORES = 8
HPC = HEADS // (NC_CORES // B)   # 4 heads per core
DQ = HPC * HD                    # 256 per-core head dims
SCALE = HD ** -0.5

NQB = 1024                       # query-block (free dim of scores psum)
P = 128

_BUILD_CACHE = {}


def build_mha(trace_friendly=False):
    key = "mha"
    if key in _BUILD_CACHE:
        return _BUILD_CACHE[key]
    nc = bacc.Bacc("TRN2", target_bir_lowering=False, debug=False,
                   num_devices=NC_CORES)
    xT_d = nc.dram_tensor("xT", [C, N], F32R, kind="ExternalInput").ap()
    wqT_d = nc.dram_tensor("wqT", [C, DQ], F32R, kind="ExternalInput").ap()
    wkT_d = nc.dram_tensor("wkT", [C, DQ], F32R, kind="ExternalInput").ap()
    wvT_d = nc.dram_tensor("wvT", [C, DQ], F32R, kind="ExternalInput").ap()
    bq_d = nc.dram_tensor("bq", [P, DQ // P], F32, kind="ExternalInput").ap()
    pwT_d = nc.dram_tensor("pwT", [DQ, C], F32R, kind="ExternalInput").ap()
    out_d = nc.dram_tensor("out", [N, C], F32, kind="ExternalOutput").ap()

    CK = C // P          # 8 contraction chunks
    DM = DQ // P         # 2 head-dim chunks
    NKC = N // P         # 16 key chunks
    NQBS = N // NQB      # 2 query blocks

    with tile.TileContext(nc) as tc:
        with tc.tile_pool(name="persist", bufs=1) as pp:
            QT_t = pp.tile([P, DM, N], F32R)
            KT_t = pp.tile([P, DM, N], F32R)
            V_t = pp.tile([P, NKC, HPC, HD + 1], F32R)
            AT_t = pp.tile([P, DM, N], F32R)
            pwT_t = pp.tile([P, DM, C], F32R)
            bq_t = pp.tile([P, DM], F32)
            ones32 = pp.tile([P, 1], F32)
            nc.vector.memset(ones32[:], 1.0)
            nc.sync.dma_start(pwT_t[:], pwT_d.rearrange("(m p) e -> p m e", p=P))
            nc.sync.dma_start(bq_t[:], bq_d)

            # ---------------- Phase 1: QKV projections ----------------
            with tc.tile_pool(name="xw", bufs=1) as xw, \
                 tc.tile_pool(name="qkv_ps", bufs=4, space="PSUM") as qps:
                xT_t = xw.tile([P, CK, N], F32R)
                wqT_t = xw.tile([P, CK, DQ], F32R)
                wkT_t = xw.tile([P, CK, DQ], F32R)
                wvT_t = xw.tile([P, CK, DQ], F32R)
                nc.sync.dma_start(xT_t[:], xT_d.rearrange("(k p) n -> p k n", p=P))
                nc.sync.dma_start(wqT_t[:], wqT_d.rearrange("(k p) d -> p k d", p=P))
                nc.sync.dma_start(wkT_t[:], wkT_d.rearrange("(k p) d -> p k d", p=P))
                nc.sync.dma_start(wvT_t[:], wvT_d.rearrange("(k p) d -> p k d", p=P))

                # QT / KT: [dq, n] transposed layout
                for m in range(DM):
                    for j in range(N // 512):
                        q_ps = qps.tile([P, 512], F32, tag="q")
                        for c in range(CK):
                            nc.tensor.matmul(
                                q_ps[:], wqT_t[:, c, m * P:(m + 1) * P],
                                xT_t[:, c, j * 512:(j + 1) * 512],
                                start=(c == 0), stop=(c == CK - 1))
                        nc.vector.tensor_scalar_add(
                            QT_t[:, m, j * 512:(j + 1) * 512], q_ps[:],
                            bq_t[:, m:m + 1])
                        k_ps = qps.tile([P, 512], F32, tag="q")
                        for c in range(CK):
                            nc.tensor.matmul(
                                k_ps[:], wkT_t[:, c, m * P:(m + 1) * P],
                                xT_t[:, c, j * 512:(j + 1) * 512],
                                start=(c == 0), stop=(c == CK - 1))
                        nc.vector.tensor_copy(
                            KT_t[:, m, j * 512:(j + 1) * 512], k_ps[:])

                # V natural layout [n, dv], packed per head with ones column
                for nk in range(NKC):
                    v_ps = qps.tile([P, DQ], F32, tag="v")
                    for c in range(CK):
                        nc.tensor.matmul(
                            v_ps[:], xT_t[:, c, nk * P:(nk + 1) * P],
                            wvT_t[:, c, :],
                            start=(c == 0), stop=(c == CK - 1))
                    nc.vector.tensor_copy(
                        V_t[:, nk, :, 0:HD],
                        v_ps[:].rearrange("p (h d) -> p h d", h=HPC))
                    for h in range(HPC):
                        nc.vector.tensor_copy(V_t[:, nk, h, HD:HD + 1], ones32[:])

            # ---------------- Phase 2: attention ----------------
            with tc.tile_pool(name="et", bufs=4) as ep, \
                 tc.tile_pool(name="sm", bufs=2) as smp, \
                 tc.tile_pool(name="s_ps", bufs=2, space="PSUM") as sps, \
                 tc.tile_pool(name="o_ps", bufs=2, space="PSUM") as ops:
                for h in range(HPC):
                    m = h // 2
                    po = (h % 2) * HD
                    for qb in range(NQBS):
                        o_ps = ops.tile([HD + 1, NQB], F32, tag="o")
                        for ck in range(NKC):
                            s_ps = sps.tile([P, NQB], F32, tag="s")
                            for j in range(NQB // 512):
                                nc.tensor.matmul(
                                    s_ps[:, j * 512:(j + 1) * 512],
                                    KT_t[po:po + HD, m, ck * P:(ck + 1) * P],
                                    QT_t[po:po + HD, m,
                                         qb * NQB + j * 512:qb * NQB + (j + 1) * 512],
                                    start=True, stop=True)
                            e_t = ep.tile([P, NQB], F32R, tag="e")
                            nc.scalar.activation(
                                e_t[:], s_ps[:], mybir.ActivationFunctionType.Exp)
                            for j in range(NQB // 512):
                                nc.tensor.matmul(
                                    o_ps[:, j * 512:(j + 1) * 512],
                                    V_t[:, ck, h, :],
                                    e_t[:, j * 512:(j + 1) * 512],
                                    start=(ck == 0), stop=(ck == NKC - 1))
                        recip_t = smp.tile([1, NQB], F32, tag="r")
                        nc.vector.reciprocal(recip_t[:], o_ps[HD:HD + 1, :])
                        rb_t = smp.tile([HD, NQB], F32, tag="rb")
                        nc.gpsimd.partition_broadcast(rb_t[:], recip_t[:])
                        nc.vector.tensor_tensor(
                            out=AT_t[po:po + HD, m, qb * NQB:(qb + 1) * NQB],
                            in0=o_ps[0:HD, :], in1=rb_t[:],
                            op=mybir.AluOpType.mult)

            # ---------------- Phase 3: output projection ----------------
            with tc.tile_pool(name="ot", bufs=3) as op_sb, \
                 tc.tile_pool(name="p_ps", bufs=4, space="PSUM") as pps:
                for nk in range(NKC):
                    out_t = op_sb.tile([P, C], F32, tag="out")
                    for e in range(C // 512):
                        p_ps = pps.tile([P, 512], F32, tag="p")
                        for m in range(DM):
                            nc.tensor.matmul(
                                p_ps[:], AT_t[:, m, nk * P:(nk + 1) * P],
                                pwT_t[:, m, e * 512:(e + 1) * 512],
                                start=(m == 0), stop=(m == DM - 1))
                        nc.vector.tensor_copy(
                            out_t[:, e * 512:(e + 1) * 512], p_ps[:])
                    nc.sync.dma_start(out_d[nk * P:(nk + 1) * P, :], out_t[:])
    nc.compile()
    _BUILD_CACHE[key] = nc
    return nc


def prep_core_inputs(x, qkv_w, qkv_b, proj_w):
    """Build the per-core input maps (host-side sharding)."""
    wq = qkv_w[0 * C:1 * C].reshape(HEADS, HD, C)
    wk = qkv_w[1 * C:2 * C].reshape(HEADS, HD, C)
    wv = qkv_w[2 * C:3 * C].reshape(HEADS, HD, C)
    bq = qkv_b[0 * C:1 * C].reshape(HEADS, HD)
    xT = [np.ascontiguousarray(x[b].T) for b in range(B)]

    in_maps = []
    for core in range(NC_CORES):
        b = core // (NC_CORES // B)
        hg = core % (NC_CORES // B)
        hs = slice(hg * HPC, (hg + 1) * HPC)
        wqT = np.ascontiguousarray(
            (wq[hs].reshape(DQ, C) * np.float32(SCALE)).T)
        wkT = np.ascontiguousarray(wk[hs].reshape(DQ, C).T)
        wvT = np.ascontiguousarray(wv[hs].reshape(DQ, C).T)
        bq_c = np.ascontiguousarray(
            (bq[hs].reshape(DQ) * np.float32(SCALE)).reshape(DQ // P, P).T)
        cols = np.arange(hg * HPC * HD, (hg + 1) * HPC * HD)
        pwT = np.ascontiguousarray(proj_w[:, cols].T)
        in_maps.append({
            "xT": xT[b], "wqT": wqT, "wkT": wkT, "wvT": wvT,
            "bq": bq_c, "pwT": pwT,
        })
    return in_maps


def kernel(x, qkv_w, qkv_b, proj_w, proj_b, _trace=False, _trace_kwargs=None):
    x = np.asarray(x, dtype=np.float32)
    qkv_w = np.asarray(qkv_w, dtype=np.float32)
    qkv_b = np.asarray(qkv_b, dtype=np.float32)
    proj_w = np.asarray(proj_w, dtype=np.float32)
    proj_b = np.asarray(proj_b, dtype=np.float32)

    nc = build_mha()
    in_maps = prep_core_inputs(x, qkv_w, qkv_b, proj_w)
    res = run_bass_kernel_spmd(nc, in_maps, list(range(NC_CORES)),
                               trace=_trace, **(_trace_kwargs or {}))

    bv = qkv_b[2 * C:3 * C]
    bias_term = proj_w @ bv + proj_b          # exact host-side bias
    out = np.zeros((B, N, C), dtype=np.float32)
    gpb = NC_CORES // B
    for core in range(NC_CORES):
        b = core // gpb
        out[b] += res.results[core]["out"]
    out += bias_term[None, None, :]
    if _trace:
        return out, res
    return out
